# revision 22
# baseline (speedup 1.0000x reference)
"""Trainium2 Bass kernel for nn_BispectrumPool.

Math (validated vs reference):
  F = FFT_8 along the group axis. beta[k] = F1*F[k]*conj(F[1+k mod 8]).
  Due to conjugate symmetry of the real-input FFT:
    beta4=beta3, beta5=beta2, beta6=beta1, beta7=beta0 (real), Im(beta0)=0
  -> only 7 distinct nonzero features per channel:
     [beta0r, beta1r, beta1i, beta2r, beta2i, beta3r, beta3i]
  with
     beta0r = F0*(b1^2+b2^2)
     beta1  = G*conj(F2),  G = F1^2      (Gr=b1^2-b2^2, Gi=2*b1b2)
     beta2  = F2*H,        H = F1*conj(F3)
     beta3  = F4*K,        K = F1*F3
  (b1,b2)=(Re,Im)F1, (b3,b4)=F2, (b5,b6)=F3, b7=F4(real), b0=F0(real).
  feat = ln(1+relu(beta_part)); y = W_folded @ feat + bias, where the 16
  original feature columns fold onto the 7 distinct ones (cols 8,15 drop).

Distribution: pure data parallel, batch 16 -> 2 per core on 8 cores.

v2 layout: x and form-matmul consts in bf16 (halves DMA); relu writes into
per-(b,q) full-row slabs [112, 3136] (3 of 4 q's on ACT, 1 on DVE); ln runs
once per (b,q) over the whole slab (amortizes ACT fixed costs); conv reads
bf16 slabs; y accumulates into a [64, 3136] slab, one output DMA per batch.
"""

import numpy as np
import ml_dtypes

C, G = 64, 8
HWP = 56 * 56            # 3136
S = 448                  # chunk width (3136 = 7*448)
NCHUNK = HWP // S        # 7
NCORES = 8
BPC = 2                  # batches per core
NQ = 4                   # channel blocks of 16


def _form_rows():
    g = np.arange(G)
    B1 = np.cos(2 * np.pi * g / G)
    B2 = -np.sin(2 * np.pi * g / G)
    B3 = np.cos(4 * np.pi * g / G)
    B4 = -np.sin(4 * np.pi * g / G)
    B5 = np.cos(6 * np.pi * g / G)
    B6 = -np.sin(6 * np.pi * g / G)
    B7 = np.cos(np.pi * g)
    B0 = np.ones(G)
    U = np.stack([B1, B2, B1, B1, B2, B2, B1])            # 7 rows
    V = np.stack([B1, B2, B2, B5, B6, B5, B6])            # 7 rows
    # blocks 4..6 = [b3, b4, b7] so the T2 product can slice at partition 64
    # (SBUF engine access must start at a 32-aligned partition)
    A = np.stack([B0, B3, B4, B3, B3, B4, B7, B4])        # 8 rows
    return U, V, A


def _combine_mats():
    # M1 blocks: [b1^2, b2^2, b1b2, b1b5, b2b6, b2b5, b1b6]
    Wc_a = np.zeros((8, 7))
    Wc_a[0, 0] = Wc_a[0, 1] = 1              # S+
    Wc_a[1, 0], Wc_a[1, 1] = 1, -1           # Gr
    Wc_a[2, 2] = 2.0                         # Gi
    Wc_a[3, 2] = 2.0                         # Gi
    Wc_a[4, 3] = Wc_a[4, 4] = 1              # Hr
    Wc_a[5, 5], Wc_a[5, 6] = 1, -1           # Hi
    Wc_a[6, 3], Wc_a[6, 4] = 1, -1           # Kr
    Wc_a[7, 0], Wc_a[7, 1] = 1, -1           # Gr
    # Cb extended to 8 out-blocks so t2 rides in the same TT as t1:
    # blocks 4,5,6 = Hi, Hr, Ki (aligned with A = [...,b3,b4,b7,...]),
    # other blocks are zero rows -> exact-zero junk products
    Wc_b = np.zeros((8, 7))
    Wc_b[4, 5], Wc_b[4, 6] = 1, -1           # Hi
    Wc_b[5, 3] = Wc_b[5, 4] = 1              # Hr
    Wc_b[6, 5] = Wc_b[6, 6] = 1              # Ki
    # T1 blocks: [b0S+, b3Gr, b4Gi, b3Gi, b3Hr, b4Hi, b7Kr, b4Gr]
    # T2 blocks: [b3Hi, b4Hr, b7Ki]
    Wr_1 = np.zeros((7, 8))
    Wr_1[0, 0] = 1                            # beta0r
    Wr_1[1, 1] = Wr_1[1, 2] = 1               # beta1r
    Wr_1[2, 3], Wr_1[2, 7] = 1, -1            # beta1i
    Wr_1[3, 4], Wr_1[3, 5] = 1, -1            # beta2r
    Wr_1[5, 6] = 1                            # beta3r
    Wr_2 = np.zeros((7, 8))
    Wr_2[4, 4] = Wr_2[4, 5] = 1               # beta2i = b3Hi + b4Hr
    Wr_2[6, 6] = 1                            # beta3i = b7Ki
    return Wc_a, Wc_b, Wr_1, Wr_2


def _block_diag_lhsT(rows_by_block_out, n_in_blocks, blk=16, in_block_of=None,
                     coef=None):
    """lhsT[k_partition, m] for a block-structured map."""
    n_out = len(coef)
    lhsT = np.zeros((n_in_blocks * blk, n_out * blk), dtype=np.float32)
    for mb in range(n_out):
        for kb in range(n_in_blocks):
            if coef[mb][kb] != 0.0:
                for c in range(blk):
                    lhsT[kb * blk + c, mb * blk + c] = coef[mb][kb]
    return lhsT


def _build_consts():
    U, V, A = _form_rows()
    Wc_a, Wc_b, Wr_1, Wr_2 = _combine_mats()

    # form matmuls: input partitions = (16c x 8g), c-major.
    def form_lhsT(rows):
        n_out = rows.shape[0]
        lhsT = np.zeros((128, n_out * 16), dtype=np.float32)
        for j in range(n_out):
            for c in range(16):
                for g in range(G):
                    lhsT[c * G + g, j * 16 + c] = rows[j, g]
        return lhsT

    cU = form_lhsT(U)                                 # [128, 112]
    cV = form_lhsT(V)                                 # [128, 112]
    cA = form_lhsT(A)                                 # [128, 128]
    cCa = _block_diag_lhsT(None, 7, coef=Wc_a).astype(np.float32)   # [112, 128]
    cCb = _block_diag_lhsT(None, 7, coef=Wc_b).astype(np.float32)   # [112, 128]
    cR1 = _block_diag_lhsT(None, 8, coef=Wr_1).astype(np.float32)   # [128, 112]
    cR2 = _block_diag_lhsT(None, 8, coef=Wr_2).astype(np.float32)   # [128, 112]
    return cU, cV, cA, cCa, cCb, cR1, cR2


def _fold_weights(conv_w):
    w = conv_w.reshape(64, C, 16)
    W7 = np.zeros((64, C, 7), dtype=np.float64)
    W7[..., 0] = w[..., 0] + w[..., 7]
    W7[..., 1] = w[..., 1] + w[..., 6]
    W7[..., 2] = w[..., 9] + w[..., 14]
    W7[..., 3] = w[..., 2] + w[..., 5]
    W7[..., 4] = w[..., 10] + w[..., 13]
    W7[..., 5] = w[..., 3] + w[..., 4]
    W7[..., 6] = w[..., 11] + w[..., 12]
    # conv lhsT per q: [112 = (7f x 16c), 64], packed side by side -> [112, 256]
    wf = np.zeros((112, NQ * 64), dtype=np.float32)
    for q in range(NQ):
        for f in range(7):
            for cl in range(16):
                wf[f * 16 + cl, q * 64:(q + 1) * 64] = W7[:, q * 16 + cl, f]
    return wf


def _prep(x, conv_w, conv_b):
    """Host-side prep shared by kernel() and test.py: returns (consts, xr)."""
    x = np.asarray(x)
    B = x.shape[0]
    xr = np.ascontiguousarray(
        x.reshape(B, C * G, HWP).astype(np.float32))
    cU, cV, cA, cCa, cCb, cR1, cR2 = _build_consts()
    wf = _fold_weights(np.asarray(conv_w).astype(np.float64))
    bias = np.ascontiguousarray(
        np.asarray(conv_b).astype(np.float32).reshape(64, 1))
    consts = dict(cU=cU, cV=cV, cA=cA, cCa=cCa, cCb=cCb, cR1=cR1, cR2=cR2,
                  wf=wf, bias=bias)
    return consts, xr


_PROG_CACHE = {}


def _build_program(loop_n=1, unroll=False):
    import concourse.bass as bass
    import concourse.bacc as bacc
    import concourse.tile as tile
    import concourse.mybir as mybir

    f32 = mybir.dt.float32
    f32r = mybir.dt.float32r
    bf16 = mybir.dt.bfloat16
    nc = bacc.Bacc("TRN2", target_bir_lowering=False, debug=False,
                   num_devices=NCORES)

    x_d = nc.dram_tensor("x", [BPC, C * G, HWP], f32r,
                         kind="ExternalInput").ap()
    cU_d = nc.dram_tensor("cU", [128, 112], f32r, kind="ExternalInput").ap()
    cV_d = nc.dram_tensor("cV", [128, 112], f32r, kind="ExternalInput").ap()
    cA_d = nc.dram_tensor("cA", [128, 128], f32r, kind="ExternalInput").ap()
    cCa_d = nc.dram_tensor("cCa", [112, 128], f32r, kind="ExternalInput").ap()
    cCb_d = nc.dram_tensor("cCb", [112, 128], f32r, kind="ExternalInput").ap()
    cR1_d = nc.dram_tensor("cR1", [128, 112], f32r, kind="ExternalInput").ap()
    cR2_d = nc.dram_tensor("cR2", [128, 112], f32r, kind="ExternalInput").ap()
    wf_d = nc.dram_tensor("wf", [112, NQ * 64], f32r,
                          kind="ExternalInput").ap()
    bias_d = nc.dram_tensor("bias", [64, 1], f32, kind="ExternalInput").ap()
    y_d = nc.dram_tensor("y", [BPC, 64, HWP], f32, kind="ExternalOutput").ap()

    LN = mybir.ActivationFunctionType.Ln
    RELU = mybir.ActivationFunctionType.Relu
    IDENT = mybir.ActivationFunctionType.Identity
    MAX = mybir.AluOpType.max

    with tile.TileContext(nc) as tc:
        with (
            tc.tile_pool(name="consts", bufs=1) as cpool,
            tc.tile_pool(name="xin", bufs=10) as xpool,
            tc.tile_pool(name="sb", bufs=4) as sbpool,
            tc.tile_pool(name="rg", bufs=3) as rgpool,
            tc.tile_pool(name="rl", bufs=3) as rlpool,
            tc.tile_pool(name="yout", bufs=2) as ypool,
            tc.tile_pool(name="psA", bufs=1, space="PSUM") as psA,
            tc.tile_pool(name="psB", bufs=1, space="PSUM") as psB,
            tc.tile_pool(name="psC", bufs=1, space="PSUM") as psC,
            tc.tile_pool(name="psY", bufs=2, space="PSUM") as psY,
        ):
            # --- load constants once ---
            cU = cpool.tile([128, 112], f32r, tag="cU")
            cV = cpool.tile([128, 112], f32r, tag="cV")
            cA = cpool.tile([128, 128], f32r, tag="cA")
            cCa = cpool.tile([112, 128], f32r, tag="cCa")
            cCb = cpool.tile([112, 128], f32r, tag="cCb")
            cR1 = cpool.tile([128, 112], f32r, tag="cR1")
            cR2 = cpool.tile([128, 112], f32r, tag="cR2")
            wf = cpool.tile([112, NQ * 64], f32r, tag="wf")
            bias = cpool.tile([64, 1], f32, tag="bias")
            for t, d in [(cU, cU_d), (cV, cV_d), (cA, cA_d), (cCa, cCa_d),
                         (cCb, cCb_d), (cR1, cR1_d), (cR2, cR2_d),
                         (wf, wf_d), (bias, bias_d)]:
                nc.sync.dma_start(out=t[:], in_=d)

            import contextlib
            if unroll:
                loop_cm = contextlib.nullcontext()
                outer = range(loop_n)
            else:
                loop_cm = (tc.For_i(0, loop_n, 1) if loop_n > 1
                           else contextlib.nullcontext())
                outer = range(1)
            with loop_cm:
             for _u in outer:
              for b in range(BPC):
                  ys = ypool.tile([64, HWP], f32, tag="ys")
                  rgs = rls = None
                  for j in range(NCHUNK):
                      s0 = j * S
                      half = (j % 2) * S
                      last = (j == NCHUNK - 1)
                      if j % 2 == 0:
                          rgs = [rgpool.tile([112, 2 * S], bf16, tag=f"rg{q}",
                                             name=f"rg{q}")
                                 for q in range(NQ)]
                      xts = []
                      for q in range(NQ):
                          xt = xpool.tile([128, S], f32r, tag="x", name="xt")
                          nc.sync.dma_start(
                              out=xt[:], in_=x_d[b, 128 * q:128 * (q + 1),
                                                 s0:s0 + S])
                          xts.append(xt)
                      for q in range(NQ):
                          xt = xts[q]
                          # form matmuls. V and A share a 2-bank PSUM
                          # tile (V at cols 0:448 in bank 0, A at 512:960 in
                          # bank 1) so ONE ScalarE op evacuates both; the
                          # 448:512 pad is stale-but-finite PSUM.
                          pU = psA.tile([112, S], f32, tag="u")
                          pVA = psB.tile([128, 1024], f32, tag="va")
                          nc.tensor.matmul(pU[:], cU[:], xt[:])
                          nc.tensor.matmul(pVA[:112, 0:S], cV[:], xt[:])
                          nc.tensor.matmul(pVA[:, 512:512 + S], cA[:], xt[:])
                          vasb = sbpool.tile([128, 1024], f32, tag="vasb")
                          nc.scalar.copy(vasb[:, 0:512 + S],
                                         pVA[:, 0:512 + S])
                          vsb = vasb[:112, 0:S]
                          # round-1 products
                          m1 = sbpool.tile([112, S], f32r, tag="m1")
                          nc.vector.tensor_mul(m1[:], pU[:], vsb)
                          # quadratic combines share a 2-bank PSUM tile,
                          # 3D [128, 2, 512]: Ca in bank 0, Cb in bank 1
                          pCab = psC.tile([128, 2, 512], f32, tag="cab")
                          nc.tensor.matmul(pCab[:, 0, 0:S], cCa[:], m1[:])
                          nc.tensor.matmul(pCab[:, 1, 0:S], cCb[:], m1[:])
                          # round-2 products: ONE TT covers t1 and t2; the
                          # A multiplier is read twice via a 0-stride dim
                          arep = vasb[:, 512:512 + S].unsqueeze(1)
                          arep = arep.broadcast_to((128, 2, S))
                          t12 = sbpool.tile([128, 2, S], f32r, tag="t12")
                          nc.vector.tensor_mul(t12[:, :, :],
                                               pCab[:, :, 0:S], arep)
                          # beta combine
                          pR = psB.tile([112, S], f32, tag="rpre")
                          nc.tensor.matmul(pR[:], cR1[:], t12[:, 0, :],
                                           start=True, stop=False)
                          nc.tensor.matmul(pR[:], cR2[:], t12[:, 1, :],
                                           start=False, stop=True)
                          # relu into the pair-chunk slab (2 q's on DVE,
                          # 2 on ACT); ln(1+x) follows per pair
                          if q == 0:
                              nc.vector.tensor_scalar(
                                  rgs[q][:, half:half + S], pR[:],
                                  0.0, None, MAX)
                          else:
                              nc.scalar.activation(
                                  rgs[q][:, half:half + S], pR[:], RELU)
                      if j % 2 == 1 or last:
                          w = half + S
                          rls = [rlpool.tile([112, 2 * S], f32r, tag=f"rl{q}",
                                             name=f"rl{q}")
                                 for q in range(NQ)]
                          for q in range(NQ):
                              nc.scalar.activation(rls[q][:, :w],
                                                   rgs[q][:, :w], LN, bias=1.0)
                          for jj in range(j - (0 if last and j % 2 == 0
                                               else 1), j + 1):
                              ss = jj * S
                              hh = (jj % 2) * S
                              pY = psY.tile([64, S], f32, tag="y")
                              for q in range(NQ):
                                  nc.tensor.matmul(
                                      pY[:], wf[:, q * 64:(q + 1) * 64],
                                      rls[q][:, hh:hh + S],
                                      start=(q == 0), stop=(q == NQ - 1))
                              nc.scalar.activation(ys[:, ss:ss + S], pY[:],
                                                   IDENT, bias=bias[:, 0:1])
                  nc.scalar.dma_start(out=y_d[b], in_=ys[:])
    nc.compile()
    return nc


def kernel(x, conv_w, conv_b):
    from concourse.bass_utils import run_bass_kernel_spmd

    consts, xr = _prep(x, conv_w, conv_b)

    key = "prog"
    if key not in _PROG_CACHE:
        _PROG_CACHE[key] = _build_program()
    nc = _PROG_CACHE[key]

    in_maps = []
    for i in range(NCORES):
        m = dict(consts)
        m["x"] = np.ascontiguousarray(xr[i * BPC:(i + 1) * BPC])
        in_maps.append(m)

    res = run_bass_kernel_spmd(nc, in_maps, core_ids=list(range(NCORES)))
    y = np.concatenate([res.results[i]["y"] for i in range(NCORES)], axis=0)
    return np.ascontiguousarray(
        y.reshape(NCORES * BPC, 64, 56, 56).astype(np.float32))


# revision 25
# speedup vs baseline: 1.0047x; 1.0047x over previous
"""Trainium2 Bass kernel for nn_BispectrumPool.

Math (validated vs reference):
  F = FFT_8 along the group axis. beta[k] = F1*F[k]*conj(F[1+k mod 8]).
  Due to conjugate symmetry of the real-input FFT:
    beta4=beta3, beta5=beta2, beta6=beta1, beta7=beta0 (real), Im(beta0)=0
  -> only 7 distinct nonzero features per channel:
     [beta0r, beta1r, beta1i, beta2r, beta2i, beta3r, beta3i]
  with
     beta0r = F0*(b1^2+b2^2)
     beta1  = G*conj(F2),  G = F1^2      (Gr=b1^2-b2^2, Gi=2*b1b2)
     beta2  = F2*H,        H = F1*conj(F3)
     beta3  = F4*K,        K = F1*F3
  (b1,b2)=(Re,Im)F1, (b3,b4)=F2, (b5,b6)=F3, b7=F4(real), b0=F0(real).
  feat = ln(1+relu(beta_part)); y = W_folded @ feat + bias, where the 16
  original feature columns fold onto the 7 distinct ones (cols 8,15 drop).

Distribution: pure data parallel, batch 16 -> 2 per core on 8 cores.

v2 layout: x and form-matmul consts in bf16 (halves DMA); relu writes into
per-(b,q) full-row slabs [112, 3136] (3 of 4 q's on ACT, 1 on DVE); ln runs
once per (b,q) over the whole slab (amortizes ACT fixed costs); conv reads
bf16 slabs; y accumulates into a [64, 3136] slab, one output DMA per batch.
"""

import numpy as np
import ml_dtypes

C, G = 64, 8
HWP = 56 * 56            # 3136
S = 448                  # chunk width (3136 = 7*448)
NCHUNK = HWP // S        # 7
NCORES = 8
BPC = 2                  # batches per core
NQ = 4                   # channel blocks of 16


def _form_rows():
    g = np.arange(G)
    B1 = np.cos(2 * np.pi * g / G)
    B2 = -np.sin(2 * np.pi * g / G)
    B3 = np.cos(4 * np.pi * g / G)
    B4 = -np.sin(4 * np.pi * g / G)
    B5 = np.cos(6 * np.pi * g / G)
    B6 = -np.sin(6 * np.pi * g / G)
    B7 = np.cos(np.pi * g)
    B0 = np.ones(G)
    U = np.stack([B1, B2, B1, B1, B2, B2, B1])            # 7 rows
    V = np.stack([B1, B2, B2, B5, B6, B5, B6])            # 7 rows
    # blocks 4..6 = [b3, b4, b7] so the T2 product can slice at partition 64
    # (SBUF engine access must start at a 32-aligned partition)
    A = np.stack([B0, B3, B4, B3, B3, B4, B7, B4])        # 8 rows
    return U, V, A


def _combine_mats():
    # M1 blocks: [b1^2, b2^2, b1b2, b1b5, b2b6, b2b5, b1b6]
    Wc_a = np.zeros((8, 7))
    Wc_a[0, 0] = Wc_a[0, 1] = 1              # S+
    Wc_a[1, 0], Wc_a[1, 1] = 1, -1           # Gr
    Wc_a[2, 2] = 2.0                         # Gi
    Wc_a[3, 2] = 2.0                         # Gi
    Wc_a[4, 3] = Wc_a[4, 4] = 1              # Hr
    Wc_a[5, 5], Wc_a[5, 6] = 1, -1           # Hi
    Wc_a[6, 3], Wc_a[6, 4] = 1, -1           # Kr
    Wc_a[7, 0], Wc_a[7, 1] = 1, -1           # Gr
    Wc_b = np.zeros((3, 7))
    Wc_b[0, 5], Wc_b[0, 6] = 1, -1           # Hi
    Wc_b[1, 3] = Wc_b[1, 4] = 1              # Hr
    Wc_b[2, 5] = Wc_b[2, 6] = 1              # Ki
    # T1 blocks: [b0S+, b3Gr, b4Gi, b3Gi, b3Hr, b4Hi, b7Kr, b4Gr]
    # T2 blocks: [b3Hi, b4Hr, b7Ki]
    Wr_1 = np.zeros((7, 8))
    Wr_1[0, 0] = 1                            # beta0r
    Wr_1[1, 1] = Wr_1[1, 2] = 1               # beta1r
    Wr_1[2, 3], Wr_1[2, 7] = 1, -1            # beta1i
    Wr_1[3, 4], Wr_1[3, 5] = 1, -1            # beta2r
    Wr_1[5, 6] = 1                            # beta3r
    Wr_2 = np.zeros((7, 3))
    Wr_2[4, 0] = Wr_2[4, 1] = 1               # beta2i
    Wr_2[6, 2] = 1                            # beta3i
    return Wc_a, Wc_b, Wr_1, Wr_2


def _block_diag_lhsT(rows_by_block_out, n_in_blocks, blk=16, in_block_of=None,
                     coef=None):
    """lhsT[k_partition, m] for a block-structured map."""
    n_out = len(coef)
    lhsT = np.zeros((n_in_blocks * blk, n_out * blk), dtype=np.float32)
    for mb in range(n_out):
        for kb in range(n_in_blocks):
            if coef[mb][kb] != 0.0:
                for c in range(blk):
                    lhsT[kb * blk + c, mb * blk + c] = coef[mb][kb]
    return lhsT


def _build_consts():
    U, V, A = _form_rows()
    Wc_a, Wc_b, Wr_1, Wr_2 = _combine_mats()

    # form matmuls: input partitions = (16c x 8g), c-major.
    def form_lhsT(rows):
        n_out = rows.shape[0]
        lhsT = np.zeros((128, n_out * 16), dtype=np.float32)
        for j in range(n_out):
            for c in range(16):
                for g in range(G):
                    lhsT[c * G + g, j * 16 + c] = rows[j, g]
        return lhsT

    cU = form_lhsT(U)                                 # [128, 112]
    cV = form_lhsT(V)                                 # [128, 112]
    cA = form_lhsT(A)                                 # [128, 128]
    cCa = _block_diag_lhsT(None, 7, coef=Wc_a).astype(np.float32)   # [112, 128]
    cCb = _block_diag_lhsT(None, 7, coef=Wc_b).astype(np.float32)   # [112, 48]
    cR1 = _block_diag_lhsT(None, 8, coef=Wr_1).astype(np.float32)   # [128, 112]
    cR2 = _block_diag_lhsT(None, 3, coef=Wr_2).astype(np.float32)   # [48, 112]
    return cU, cV, cA, cCa, cCb, cR1, cR2


def _fold_weights(conv_w):
    w = conv_w.reshape(64, C, 16)
    W7 = np.zeros((64, C, 7), dtype=np.float64)
    W7[..., 0] = w[..., 0] + w[..., 7]
    W7[..., 1] = w[..., 1] + w[..., 6]
    W7[..., 2] = w[..., 9] + w[..., 14]
    W7[..., 3] = w[..., 2] + w[..., 5]
    W7[..., 4] = w[..., 10] + w[..., 13]
    W7[..., 5] = w[..., 3] + w[..., 4]
    W7[..., 6] = w[..., 11] + w[..., 12]
    # conv lhsT per q: [112 = (7f x 16c), 64], packed side by side -> [112, 256]
    wf = np.zeros((112, NQ * 64), dtype=np.float32)
    for q in range(NQ):
        for f in range(7):
            for cl in range(16):
                wf[f * 16 + cl, q * 64:(q + 1) * 64] = W7[:, q * 16 + cl, f]
    return wf


def _prep(x, conv_w, conv_b):
    """Host-side prep shared by kernel() and test.py: returns (consts, xr)."""
    x = np.asarray(x)
    B = x.shape[0]
    xr = np.ascontiguousarray(
        x.reshape(B, C * G, HWP).astype(np.float32))
    cU, cV, cA, cCa, cCb, cR1, cR2 = _build_consts()
    wf = _fold_weights(np.asarray(conv_w).astype(np.float64))
    bias = np.ascontiguousarray(
        np.asarray(conv_b).astype(np.float32).reshape(64, 1))
    consts = dict(cU=cU, cV=cV, cA=cA, cCa=cCa, cCb=cCb, cR1=cR1, cR2=cR2,
                  wf=wf, bias=bias)
    return consts, xr


_PROG_CACHE = {}


def _build_program(loop_n=1, unroll=False):
    import concourse.bass as bass
    import concourse.bacc as bacc
    import concourse.tile as tile
    import concourse.mybir as mybir

    f32 = mybir.dt.float32
    f32r = mybir.dt.float32r
    bf16 = mybir.dt.bfloat16
    nc = bacc.Bacc("TRN2", target_bir_lowering=False, debug=False,
                   num_devices=NCORES)

    x_d = nc.dram_tensor("x", [BPC, C * G, HWP], f32r,
                         kind="ExternalInput").ap()
    cU_d = nc.dram_tensor("cU", [128, 112], f32r, kind="ExternalInput").ap()
    cV_d = nc.dram_tensor("cV", [128, 112], f32r, kind="ExternalInput").ap()
    cA_d = nc.dram_tensor("cA", [128, 128], f32r, kind="ExternalInput").ap()
    cCa_d = nc.dram_tensor("cCa", [112, 128], f32r, kind="ExternalInput").ap()
    cCb_d = nc.dram_tensor("cCb", [112, 48], f32r, kind="ExternalInput").ap()
    cR1_d = nc.dram_tensor("cR1", [128, 112], f32r, kind="ExternalInput").ap()
    cR2_d = nc.dram_tensor("cR2", [48, 112], f32r, kind="ExternalInput").ap()
    wf_d = nc.dram_tensor("wf", [112, NQ * 64], f32r,
                          kind="ExternalInput").ap()
    bias_d = nc.dram_tensor("bias", [64, 1], f32, kind="ExternalInput").ap()
    y_d = nc.dram_tensor("y", [BPC, 64, HWP], f32, kind="ExternalOutput").ap()

    LN = mybir.ActivationFunctionType.Ln
    RELU = mybir.ActivationFunctionType.Relu
    IDENT = mybir.ActivationFunctionType.Identity
    MAX = mybir.AluOpType.max

    with tile.TileContext(nc) as tc:
        with (
            tc.tile_pool(name="consts", bufs=1) as cpool,
            tc.tile_pool(name="xin", bufs=10) as xpool,
            tc.tile_pool(name="sb", bufs=6) as sbpool,
            tc.tile_pool(name="rg", bufs=3) as rgpool,
            tc.tile_pool(name="rl", bufs=3) as rlpool,
            tc.tile_pool(name="yout", bufs=2) as ypool,
            tc.tile_pool(name="psA", bufs=1, space="PSUM") as psA,
            tc.tile_pool(name="psB", bufs=1, space="PSUM") as psB,
            tc.tile_pool(name="psC", bufs=1, space="PSUM") as psC,
            tc.tile_pool(name="psY", bufs=2, space="PSUM") as psY,
        ):
            # --- load constants once ---
            cU = cpool.tile([128, 112], f32r, tag="cU")
            cV = cpool.tile([128, 112], f32r, tag="cV")
            cA = cpool.tile([128, 128], f32r, tag="cA")
            cCa = cpool.tile([112, 128], f32r, tag="cCa")
            cCb = cpool.tile([112, 48], f32r, tag="cCb")
            cR1 = cpool.tile([128, 112], f32r, tag="cR1")
            cR2 = cpool.tile([48, 112], f32r, tag="cR2")
            wf = cpool.tile([112, NQ * 64], f32r, tag="wf")
            bias = cpool.tile([64, 1], f32, tag="bias")
            for t, d in [(cU, cU_d), (cV, cV_d), (cA, cA_d), (cCa, cCa_d),
                         (cCb, cCb_d), (cR1, cR1_d), (cR2, cR2_d),
                         (wf, wf_d), (bias, bias_d)]:
                nc.sync.dma_start(out=t[:], in_=d)

            import contextlib
            if unroll:
                loop_cm = contextlib.nullcontext()
                outer = range(loop_n)
            else:
                loop_cm = (tc.For_i(0, loop_n, 1) if loop_n > 1
                           else contextlib.nullcontext())
                outer = range(1)
            with loop_cm:
             for _u in outer:
              for b in range(BPC):
                  ys = ypool.tile([64, HWP], f32, tag="ys")
                  rgs = rls = None
                  for j in range(NCHUNK):
                      s0 = j * S
                      half = (j % 2) * S
                      last = (j == NCHUNK - 1)
                      if j % 2 == 0:
                          rgs = [rgpool.tile([112, 2 * S], bf16, tag=f"rg{q}",
                                             name=f"rg{q}")
                                 for q in range(NQ)]
                      xts = []
                      for q in range(NQ):
                          xt = xpool.tile([128, S], f32r, tag="x", name="xt")
                          nc.sync.dma_start(
                              out=xt[:], in_=x_d[b, 128 * q:128 * (q + 1),
                                                 s0:s0 + S])
                          xts.append(xt)
                      for q in range(NQ):
                          xt = xts[q]
                          # form matmuls. V and A share a 2-bank PSUM
                          # tile (V at cols 0:448 in bank 0, A at 512:960 in
                          # bank 1) so ONE ScalarE op evacuates both; the
                          # 448:512 pad is stale-but-finite PSUM.
                          pU = psA.tile([112, S], f32, tag="u")
                          pVA = psB.tile([128, 1024], f32, tag="va")
                          nc.tensor.matmul(pU[:], cU[:], xt[:])
                          nc.tensor.matmul(pVA[:112, 0:S], cV[:], xt[:])
                          nc.tensor.matmul(pVA[:, 512:512 + S], cA[:], xt[:])
                          vasb = sbpool.tile([128, 1024], f32, tag="vasb")
                          nc.scalar.copy(vasb[:, 0:512 + S],
                                         pVA[:, 0:512 + S])
                          vsb = vasb[:112, 0:S]
                          asb = vasb[:, 512:512 + S]
                          # round-1 products
                          m1 = sbpool.tile([112, S], f32r, tag="m1")
                          nc.vector.tensor_mul(m1[:], pU[:], vsb)
                          # quadratic combines
                          pCa = psC.tile([128, S], f32, tag="ca")
                          pCb = psC.tile([48, S], f32, tag="cb")
                          nc.tensor.matmul(pCa[:], cCa[:], m1[:])
                          nc.tensor.matmul(pCb[:], cCb[:], m1[:])
                          # round-2 products
                          t1 = sbpool.tile([128, S], f32r, tag="t1")
                          t2 = sbpool.tile([48, S], f32r, tag="t2")
                          nc.vector.tensor_mul(t1[:], asb, pCa[:])
                          nc.vector.tensor_mul(
                              t2[:], vasb[64:112, 512:512 + S], pCb[:])
                          # beta combine
                          pR = psB.tile([112, S], f32, tag="rpre")
                          nc.tensor.matmul(pR[:], cR1[:], t1[:],
                                           start=True, stop=False)
                          nc.tensor.matmul(pR[:], cR2[:], t2[:],
                                           start=False, stop=True)
                          # relu into the pair-chunk slab (2 q's on DVE,
                          # 2 on ACT); ln(1+x) follows per pair
                          if q == 0:
                              nc.vector.tensor_scalar(
                                  rgs[q][:, half:half + S], pR[:],
                                  0.0, None, MAX)
                          else:
                              nc.scalar.activation(
                                  rgs[q][:, half:half + S], pR[:], RELU)
                      if j % 2 == 1 or last:
                          w = half + S
                          rls = [rlpool.tile([112, 2 * S], f32r, tag=f"rl{q}",
                                             name=f"rl{q}")
                                 for q in range(NQ)]
                          for q in range(NQ):
                              nc.scalar.activation(rls[q][:, :w],
                                                   rgs[q][:, :w], LN, bias=1.0)
                          for jj in range(j - (0 if last and j % 2 == 0
                                               else 1), j + 1):
                              ss = jj * S
                              hh = (jj % 2) * S
                              pY = psY.tile([64, S], f32, tag="y")
                              for q in range(NQ):
                                  nc.tensor.matmul(
                                      pY[:], wf[:, q * 64:(q + 1) * 64],
                                      rls[q][:, hh:hh + S],
                                      start=(q == 0), stop=(q == NQ - 1))
                              nc.scalar.activation(ys[:, ss:ss + S], pY[:],
                                                   IDENT, bias=bias[:, 0:1])
                  nc.scalar.dma_start(out=y_d[b], in_=ys[:])
    nc.compile()
    return nc


def kernel(x, conv_w, conv_b):
    from concourse.bass_utils import run_bass_kernel_spmd

    consts, xr = _prep(x, conv_w, conv_b)

    key = "prog"
    if key not in _PROG_CACHE:
        _PROG_CACHE[key] = _build_program()
    nc = _PROG_CACHE[key]

    in_maps = []
    for i in range(NCORES):
        m = dict(consts)
        m["x"] = np.ascontiguousarray(xr[i * BPC:(i + 1) * BPC])
        in_maps.append(m)

    res = run_bass_kernel_spmd(nc, in_maps, core_ids=list(range(NCORES)))
    y = np.concatenate([res.results[i]["y"] for i in range(NCORES)], axis=0)
    return np.ascontiguousarray(
        y.reshape(NCORES * BPC, 64, 56, 56).astype(np.float32))


# revision 27
# speedup vs baseline: 1.0417x; 1.0368x over previous
"""Trainium2 Bass kernel for nn_BispectrumPool.

Math (validated vs reference):
  F = FFT_8 along the group axis. beta[k] = F1*F[k]*conj(F[1+k mod 8]).
  Due to conjugate symmetry of the real-input FFT:
    beta4=beta3, beta5=beta2, beta6=beta1, beta7=beta0 (real), Im(beta0)=0
  -> only 7 distinct nonzero features per channel:
     [beta0r, beta1r, beta1i, beta2r, beta2i, beta3r, beta3i]
  with
     beta0r = F0*(b1^2+b2^2)
     beta1  = G*conj(F2),  G = F1^2      (Gr=b1^2-b2^2, Gi=2*b1b2)
     beta2  = F2*H,        H = F1*conj(F3)
     beta3  = F4*K,        K = F1*F3
  (b1,b2)=(Re,Im)F1, (b3,b4)=F2, (b5,b6)=F3, b7=F4(real), b0=F0(real).
  feat = ln(1+relu(beta_part)); y = W_folded @ feat + bias, where the 16
  original feature columns fold onto the 7 distinct ones (cols 8,15 drop).

Distribution: pure data parallel, batch 16 -> 2 per core on 8 cores.

v2 layout: x and form-matmul consts in bf16 (halves DMA); relu writes into
per-(b,q) full-row slabs [112, 3136] (3 of 4 q's on ACT, 1 on DVE); ln runs
once per (b,q) over the whole slab (amortizes ACT fixed costs); conv reads
bf16 slabs; y accumulates into a [64, 3136] slab, one output DMA per batch.
"""

import numpy as np
import ml_dtypes

C, G = 64, 8
HWP = 56 * 56            # 3136
S = 448                  # chunk width (3136 = 7*448)
NCHUNK = HWP // S        # 7
NCORES = 8
BPC = 2                  # batches per core
NQ = 4                   # channel blocks of 16


def _form_rows():
    g = np.arange(G)
    B1 = np.cos(2 * np.pi * g / G)
    B2 = -np.sin(2 * np.pi * g / G)
    B3 = np.cos(4 * np.pi * g / G)
    B4 = -np.sin(4 * np.pi * g / G)
    B5 = np.cos(6 * np.pi * g / G)
    B6 = -np.sin(6 * np.pi * g / G)
    B7 = np.cos(np.pi * g)
    B0 = np.ones(G)
    U = np.stack([B1, B2, B1, B1, B2, B2, B1])            # 7 rows
    V = np.stack([B1, B2, B2, B5, B6, B5, B6])            # 7 rows
    # blocks 4..6 = [b3, b4, b7] so the T2 product can slice at partition 64
    # (SBUF engine access must start at a 32-aligned partition)
    A = np.stack([B0, B3, B4, B3, B3, B4, B7, B4])        # 8 rows
    return U, V, A


def _combine_mats():
    # M1 blocks: [b1^2, b2^2, b1b2, b1b5, b2b6, b2b5, b1b6]
    Wc_a = np.zeros((8, 7))
    Wc_a[0, 0] = Wc_a[0, 1] = 1              # S+
    Wc_a[1, 0], Wc_a[1, 1] = 1, -1           # Gr
    Wc_a[2, 2] = 2.0                         # Gi
    Wc_a[3, 2] = 2.0                         # Gi
    Wc_a[4, 3] = Wc_a[4, 4] = 1              # Hr
    Wc_a[5, 5], Wc_a[5, 6] = 1, -1           # Hi
    Wc_a[6, 3], Wc_a[6, 4] = 1, -1           # Kr
    Wc_a[7, 0], Wc_a[7, 1] = 1, -1           # Gr
    Wc_b = np.zeros((3, 7))
    Wc_b[0, 5], Wc_b[0, 6] = 1, -1           # Hi
    Wc_b[1, 3] = Wc_b[1, 4] = 1              # Hr
    Wc_b[2, 5] = Wc_b[2, 6] = 1              # Ki
    # T1 blocks: [b0S+, b3Gr, b4Gi, b3Gi, b3Hr, b4Hi, b7Kr, b4Gr]
    # T2 blocks: [b3Hi, b4Hr, b7Ki]
    Wr_1 = np.zeros((7, 8))
    Wr_1[0, 0] = 1                            # beta0r
    Wr_1[1, 1] = Wr_1[1, 2] = 1               # beta1r
    Wr_1[2, 3], Wr_1[2, 7] = 1, -1            # beta1i
    Wr_1[3, 4], Wr_1[3, 5] = 1, -1            # beta2r
    Wr_1[5, 6] = 1                            # beta3r
    Wr_2 = np.zeros((7, 3))
    Wr_2[4, 0] = Wr_2[4, 1] = 1               # beta2i
    Wr_2[6, 2] = 1                            # beta3i
    return Wc_a, Wc_b, Wr_1, Wr_2


def _block_diag_lhsT(rows_by_block_out, n_in_blocks, blk=16, in_block_of=None,
                     coef=None):
    """lhsT[k_partition, m] for a block-structured map."""
    n_out = len(coef)
    lhsT = np.zeros((n_in_blocks * blk, n_out * blk), dtype=np.float32)
    for mb in range(n_out):
        for kb in range(n_in_blocks):
            if coef[mb][kb] != 0.0:
                for c in range(blk):
                    lhsT[kb * blk + c, mb * blk + c] = coef[mb][kb]
    return lhsT


def _build_consts():
    U, V, A = _form_rows()
    Wc_a, Wc_b, Wr_1, Wr_2 = _combine_mats()

    # form matmuls: input partitions = (16c x 8g), c-major.
    def form_lhsT(rows):
        n_out = rows.shape[0]
        lhsT = np.zeros((128, n_out * 16), dtype=np.float32)
        for j in range(n_out):
            for c in range(16):
                for g in range(G):
                    lhsT[c * G + g, j * 16 + c] = rows[j, g]
        return lhsT

    cU = form_lhsT(U)                                 # [128, 112]
    cV = form_lhsT(V)                                 # [128, 112]
    cA = form_lhsT(A)                                 # [128, 128]
    cCa = _block_diag_lhsT(None, 7, coef=Wc_a).astype(np.float32)   # [112, 128]
    cCb = _block_diag_lhsT(None, 7, coef=Wc_b).astype(np.float32)   # [112, 48]
    cR1 = _block_diag_lhsT(None, 8, coef=Wr_1).astype(np.float32)   # [128, 112]
    cR2 = _block_diag_lhsT(None, 3, coef=Wr_2).astype(np.float32)   # [48, 112]
    return cU, cV, cA, cCa, cCb, cR1, cR2


def _fold_weights(conv_w):
    w = conv_w.reshape(64, C, 16)
    W7 = np.zeros((64, C, 7), dtype=np.float64)
    W7[..., 0] = w[..., 0] + w[..., 7]
    W7[..., 1] = w[..., 1] + w[..., 6]
    W7[..., 2] = w[..., 9] + w[..., 14]
    W7[..., 3] = w[..., 2] + w[..., 5]
    W7[..., 4] = w[..., 10] + w[..., 13]
    W7[..., 5] = w[..., 3] + w[..., 4]
    W7[..., 6] = w[..., 11] + w[..., 12]
    # conv lhsT per q: [112 = (7f x 16c), 64], packed side by side -> [112, 256]
    wf = np.zeros((112, NQ * 64), dtype=np.float32)
    for q in range(NQ):
        for f in range(7):
            for cl in range(16):
                wf[f * 16 + cl, q * 64:(q + 1) * 64] = W7[:, q * 16 + cl, f]
    return wf


def _prep(x, conv_w, conv_b):
    """Host-side prep shared by kernel() and test.py: returns (consts, xr)."""
    x = np.asarray(x)
    B = x.shape[0]
    xr = np.ascontiguousarray(
        x.reshape(B, C * G, HWP).astype(np.float32))
    cU, cV, cA, cCa, cCb, cR1, cR2 = _build_consts()
    wf = _fold_weights(np.asarray(conv_w).astype(np.float64))
    bias = np.ascontiguousarray(
        np.asarray(conv_b).astype(np.float32).reshape(64, 1))
    consts = dict(cU=cU, cV=cV, cA=cA, cCa=cCa, cCb=cCb, cR1=cR1, cR2=cR2,
                  wf=wf, bias=bias)
    return consts, xr


_PROG_CACHE = {}


def _build_program(loop_n=1, unroll=False):
    import concourse.bass as bass
    import concourse.bacc as bacc
    import concourse.tile as tile
    import concourse.mybir as mybir

    f32 = mybir.dt.float32
    f32r = mybir.dt.float32r
    bf16 = mybir.dt.bfloat16
    nc = bacc.Bacc("TRN2", target_bir_lowering=False, debug=False,
                   num_devices=NCORES)

    x_d = nc.dram_tensor("x", [BPC, C * G, HWP], f32r,
                         kind="ExternalInput").ap()
    cU_d = nc.dram_tensor("cU", [128, 112], f32r, kind="ExternalInput").ap()
    cV_d = nc.dram_tensor("cV", [128, 112], f32r, kind="ExternalInput").ap()
    cA_d = nc.dram_tensor("cA", [128, 128], f32r, kind="ExternalInput").ap()
    cCa_d = nc.dram_tensor("cCa", [112, 128], f32r, kind="ExternalInput").ap()
    cCb_d = nc.dram_tensor("cCb", [112, 48], f32r, kind="ExternalInput").ap()
    cR1_d = nc.dram_tensor("cR1", [128, 112], f32r, kind="ExternalInput").ap()
    cR2_d = nc.dram_tensor("cR2", [48, 112], f32r, kind="ExternalInput").ap()
    wf_d = nc.dram_tensor("wf", [112, NQ * 64], f32r,
                          kind="ExternalInput").ap()
    bias_d = nc.dram_tensor("bias", [64, 1], f32, kind="ExternalInput").ap()
    y_d = nc.dram_tensor("y", [BPC, 64, HWP], f32, kind="ExternalOutput").ap()

    LN = mybir.ActivationFunctionType.Ln
    RELU = mybir.ActivationFunctionType.Relu
    IDENT = mybir.ActivationFunctionType.Identity
    MAX = mybir.AluOpType.max

    with tile.TileContext(nc) as tc:
        with (
            tc.tile_pool(name="consts", bufs=1) as cpool,
            tc.tile_pool(name="xin", bufs=10) as xpool,
            tc.tile_pool(name="sb", bufs=4) as sbpool,
            tc.tile_pool(name="rg", bufs=3) as rgpool,
            tc.tile_pool(name="rl", bufs=3) as rlpool,
            tc.tile_pool(name="yout", bufs=2) as ypool,
            tc.tile_pool(name="psA", bufs=1, space="PSUM") as psA,
            tc.tile_pool(name="psB", bufs=1, space="PSUM") as psB,
            tc.tile_pool(name="psC", bufs=1, space="PSUM") as psC,
            tc.tile_pool(name="psY", bufs=2, space="PSUM") as psY,
        ):
            # --- load constants once ---
            cU = cpool.tile([128, 112], f32r, tag="cU")
            cV = cpool.tile([128, 112], f32r, tag="cV")
            cA = cpool.tile([128, 128], f32r, tag="cA")
            cCa = cpool.tile([112, 128], f32r, tag="cCa")
            cCb = cpool.tile([112, 48], f32r, tag="cCb")
            cR1 = cpool.tile([128, 112], f32r, tag="cR1")
            cR2 = cpool.tile([48, 112], f32r, tag="cR2")
            wf = cpool.tile([112, NQ * 64], f32r, tag="wf")
            bias = cpool.tile([64, 1], f32, tag="bias")
            for t, d in [(cU, cU_d), (cV, cV_d), (cA, cA_d), (cCa, cCa_d),
                         (cCb, cCb_d), (cR1, cR1_d), (cR2, cR2_d),
                         (wf, wf_d), (bias, bias_d)]:
                nc.sync.dma_start(out=t[:], in_=d)

            import contextlib
            if unroll:
                loop_cm = contextlib.nullcontext()
                outer = range(loop_n)
            else:
                loop_cm = (tc.For_i(0, loop_n, 1) if loop_n > 1
                           else contextlib.nullcontext())
                outer = range(1)
            with loop_cm:
             for _u in outer:
              for b in range(BPC):
                  ys = ypool.tile([64, HWP], f32, tag="ys")
                  rgs = rls = None
                  for j in range(NCHUNK):
                      s0 = j * S
                      half = (j % 2) * S
                      last = (j == NCHUNK - 1)
                      if j % 2 == 0:
                          rgs = [rgpool.tile([112, 2 * S], bf16, tag=f"rg{q}",
                                             name=f"rg{q}")
                                 for q in range(NQ)]
                      xts = []
                      for q in range(NQ):
                          xt = xpool.tile([128, S], f32r, tag="x", name="xt")
                          nc.sync.dma_start(
                              out=xt[:], in_=x_d[b, 128 * q:128 * (q + 1),
                                                 s0:s0 + S])
                          xts.append(xt)
                      for q in range(NQ):
                          xt = xts[q]
                          # form matmuls. V and A share a 2-bank PSUM
                          # tile (V at cols 0:448 in bank 0, A at 512:960 in
                          # bank 1) so ONE ScalarE op evacuates both; the
                          # 448:512 pad is stale-but-finite PSUM.
                          pU = psA.tile([112, S], f32, tag="u")
                          pVA = psB.tile([128, 2, 512], f32, tag="va")
                          nc.tensor.matmul(pU[:], cU[:], xt[:])
                          nc.tensor.matmul(pVA[:112, 0, 0:S], cV[:], xt[:])
                          nc.tensor.matmul(pVA[:, 1, 0:S], cA[:], xt[:])
                          vasb = sbpool.tile([128, 2, S], f32, tag="vasb")
                          nc.scalar.copy(vasb[:, :, :], pVA[:, :, 0:S])
                          vsb = vasb[:112, 0, :]
                          asb = vasb[:, 1, :]
                          # round-1 products
                          m1 = sbpool.tile([112, S], f32r, tag="m1")
                          nc.vector.tensor_mul(m1[:], pU[:], vsb)
                          # quadratic combines
                          pCa = psC.tile([128, S], f32, tag="ca")
                          pCb = psC.tile([48, S], f32, tag="cb")
                          nc.tensor.matmul(pCa[:], cCa[:], m1[:])
                          nc.tensor.matmul(pCb[:], cCb[:], m1[:])
                          # round-2 products
                          t1 = sbpool.tile([128, S], f32r, tag="t1")
                          t2 = sbpool.tile([48, S], f32r, tag="t2")
                          nc.vector.tensor_mul(t1[:], asb, pCa[:])
                          nc.vector.tensor_mul(
                              t2[:], vasb[64:112, 1, :], pCb[:])
                          # beta combine
                          pR = psB.tile([112, S], f32, tag="rpre")
                          nc.tensor.matmul(pR[:], cR1[:], t1[:],
                                           start=True, stop=False)
                          nc.tensor.matmul(pR[:], cR2[:], t2[:],
                                           start=False, stop=True)
                          # relu into the pair-chunk slab (2 q's on DVE,
                          # 2 on ACT); ln(1+x) follows per pair
                          if q == 0:
                              nc.vector.tensor_scalar(
                                  rgs[q][:, half:half + S], pR[:],
                                  0.0, None, MAX)
                          else:
                              nc.scalar.activation(
                                  rgs[q][:, half:half + S], pR[:], RELU)
                      if j % 2 == 1 or last:
                          w = half + S
                          rls = [rlpool.tile([112, 2 * S], f32r, tag=f"rl{q}",
                                             name=f"rl{q}")
                                 for q in range(NQ)]
                          for q in range(NQ):
                              nc.scalar.activation(rls[q][:, :w],
                                                   rgs[q][:, :w], LN, bias=1.0)
                          for jj in range(j - (0 if last and j % 2 == 0
                                               else 1), j + 1):
                              ss = jj * S
                              hh = (jj % 2) * S
                              pY = psY.tile([64, S], f32, tag="y")
                              for q in range(NQ):
                                  nc.tensor.matmul(
                                      pY[:], wf[:, q * 64:(q + 1) * 64],
                                      rls[q][:, hh:hh + S],
                                      start=(q == 0), stop=(q == NQ - 1))
                              nc.scalar.activation(ys[:, ss:ss + S], pY[:],
                                                   IDENT, bias=bias[:, 0:1])
                  nc.scalar.dma_start(out=y_d[b], in_=ys[:])
    nc.compile()
    return nc


def kernel(x, conv_w, conv_b):
    from concourse.bass_utils import run_bass_kernel_spmd

    consts, xr = _prep(x, conv_w, conv_b)

    key = "prog"
    if key not in _PROG_CACHE:
        _PROG_CACHE[key] = _build_program()
    nc = _PROG_CACHE[key]

    in_maps = []
    for i in range(NCORES):
        m = dict(consts)
        m["x"] = np.ascontiguousarray(xr[i * BPC:(i + 1) * BPC])
        in_maps.append(m)

    res = run_bass_kernel_spmd(nc, in_maps, core_ids=list(range(NCORES)))
    y = np.concatenate([res.results[i]["y"] for i in range(NCORES)], axis=0)
    return np.ascontiguousarray(
        y.reshape(NCORES * BPC, 64, 56, 56).astype(np.float32))


# revision 29
# speedup vs baseline: 1.1981x; 1.1502x over previous
"""Trainium2 Bass kernel for nn_BispectrumPool.

Math (validated vs reference):
  F = FFT_8 along the group axis. beta[k] = F1*F[k]*conj(F[1+k mod 8]).
  Due to conjugate symmetry of the real-input FFT:
    beta4=beta3, beta5=beta2, beta6=beta1, beta7=beta0 (real), Im(beta0)=0
  -> only 7 distinct nonzero features per channel:
     [beta0r, beta1r, beta1i, beta2r, beta2i, beta3r, beta3i]
  with
     beta0r = F0*(b1^2+b2^2)
     beta1  = G*conj(F2),  G = F1^2      (Gr=b1^2-b2^2, Gi=2*b1b2)
     beta2  = F2*H,        H = F1*conj(F3)
     beta3  = F4*K,        K = F1*F3
  (b1,b2)=(Re,Im)F1, (b3,b4)=F2, (b5,b6)=F3, b7=F4(real), b0=F0(real).
  feat = ln(1+relu(beta_part)); y = W_folded @ feat + bias, where the 16
  original feature columns fold onto the 7 distinct ones (cols 8,15 drop).

Distribution: pure data parallel, batch 16 -> 2 per core on 8 cores.

v2 layout: x and form-matmul consts in bf16 (halves DMA); relu writes into
per-(b,q) full-row slabs [112, 3136] (3 of 4 q's on ACT, 1 on DVE); ln runs
once per (b,q) over the whole slab (amortizes ACT fixed costs); conv reads
bf16 slabs; y accumulates into a [64, 3136] slab, one output DMA per batch.
"""

import numpy as np
import ml_dtypes

C, G = 64, 8
HWP = 56 * 56            # 3136
S = 448                  # chunk width (3136 = 7*448)
NCHUNK = HWP // S        # 7
NCORES = 8
BPC = 2                  # batches per core
NQ = 4                   # channel blocks of 16


def _form_rows():
    g = np.arange(G)
    B1 = np.cos(2 * np.pi * g / G)
    B2 = -np.sin(2 * np.pi * g / G)
    B3 = np.cos(4 * np.pi * g / G)
    B4 = -np.sin(4 * np.pi * g / G)
    B5 = np.cos(6 * np.pi * g / G)
    B6 = -np.sin(6 * np.pi * g / G)
    B7 = np.cos(np.pi * g)
    B0 = np.ones(G)
    U = np.stack([B1, B2, B1, B1, B2, B2, B1])            # 7 rows
    V = np.stack([B1, B2, B2, B5, B6, B5, B6])            # 7 rows
    # blocks 4..6 = [b3, b4, b7] so the T2 product can slice at partition 64
    # (SBUF engine access must start at a 32-aligned partition)
    A = np.stack([B0, B3, B4, B3, B3, B4, B7, B4])        # 8 rows
    return U, V, A


def _combine_mats():
    # M1 blocks: [b1^2, b2^2, b1b2, b1b5, b2b6, b2b5, b1b6]
    Wc_a = np.zeros((8, 7))
    Wc_a[0, 0] = Wc_a[0, 1] = 1              # S+
    Wc_a[1, 0], Wc_a[1, 1] = 1, -1           # Gr
    Wc_a[2, 2] = 2.0                         # Gi
    Wc_a[3, 2] = 2.0                         # Gi
    Wc_a[4, 3] = Wc_a[4, 4] = 1              # Hr
    Wc_a[5, 5], Wc_a[5, 6] = 1, -1           # Hi
    Wc_a[6, 3], Wc_a[6, 4] = 1, -1           # Kr
    Wc_a[7, 0], Wc_a[7, 1] = 1, -1           # Gr
    Wc_b = np.zeros((3, 7))
    Wc_b[0, 5], Wc_b[0, 6] = 1, -1           # Hi
    Wc_b[1, 3] = Wc_b[1, 4] = 1              # Hr
    Wc_b[2, 5] = Wc_b[2, 6] = 1              # Ki
    # T1 blocks: [b0S+, b3Gr, b4Gi, b3Gi, b3Hr, b4Hi, b7Kr, b4Gr]
    # T2 blocks: [b3Hi, b4Hr, b7Ki]
    Wr_1 = np.zeros((7, 8))
    Wr_1[0, 0] = 1                            # beta0r
    Wr_1[1, 1] = Wr_1[1, 2] = 1               # beta1r
    Wr_1[2, 3], Wr_1[2, 7] = 1, -1            # beta1i
    Wr_1[3, 4], Wr_1[3, 5] = 1, -1            # beta2r
    Wr_1[5, 6] = 1                            # beta3r
    Wr_2 = np.zeros((7, 3))
    Wr_2[4, 0] = Wr_2[4, 1] = 1               # beta2i
    Wr_2[6, 2] = 1                            # beta3i
    return Wc_a, Wc_b, Wr_1, Wr_2


def _block_diag_lhsT(rows_by_block_out, n_in_blocks, blk=16, in_block_of=None,
                     coef=None):
    """lhsT[k_partition, m] for a block-structured map."""
    n_out = len(coef)
    lhsT = np.zeros((n_in_blocks * blk, n_out * blk), dtype=np.float32)
    for mb in range(n_out):
        for kb in range(n_in_blocks):
            if coef[mb][kb] != 0.0:
                for c in range(blk):
                    lhsT[kb * blk + c, mb * blk + c] = coef[mb][kb]
    return lhsT


def _build_consts():
    U, V, A = _form_rows()
    Wc_a, Wc_b, Wr_1, Wr_2 = _combine_mats()

    # form matmuls: input partitions = (16c x 8g), c-major.
    def form_lhsT(rows):
        n_out = rows.shape[0]
        lhsT = np.zeros((128, n_out * 16), dtype=np.float32)
        for j in range(n_out):
            for c in range(16):
                for g in range(G):
                    lhsT[c * G + g, j * 16 + c] = rows[j, g]
        return lhsT

    cU = form_lhsT(U)                                 # [128, 112]
    cV = form_lhsT(V)                                 # [128, 112]
    cA = form_lhsT(A)                                 # [128, 128]
    cCa = _block_diag_lhsT(None, 7, coef=Wc_a).astype(np.float32)   # [112, 128]
    cCb = _block_diag_lhsT(None, 7, coef=Wc_b).astype(np.float32)   # [112, 48]
    cR1 = _block_diag_lhsT(None, 8, coef=Wr_1).astype(np.float32)   # [128, 112]
    cR2 = _block_diag_lhsT(None, 3, coef=Wr_2).astype(np.float32)   # [48, 112]
    return cU, cV, cA, cCa, cCb, cR1, cR2


def _fold_weights(conv_w):
    w = conv_w.reshape(64, C, 16)
    W7 = np.zeros((64, C, 7), dtype=np.float64)
    W7[..., 0] = w[..., 0] + w[..., 7]
    W7[..., 1] = w[..., 1] + w[..., 6]
    W7[..., 2] = w[..., 9] + w[..., 14]
    W7[..., 3] = w[..., 2] + w[..., 5]
    W7[..., 4] = w[..., 10] + w[..., 13]
    W7[..., 5] = w[..., 3] + w[..., 4]
    W7[..., 6] = w[..., 11] + w[..., 12]
    # conv lhsT per q: [112 = (7f x 16c), 64], packed side by side -> [112, 256]
    wf = np.zeros((112, NQ * 64), dtype=np.float32)
    for q in range(NQ):
        for f in range(7):
            for cl in range(16):
                wf[f * 16 + cl, q * 64:(q + 1) * 64] = W7[:, q * 16 + cl, f]
    return wf


def _prep(x, conv_w, conv_b):
    """Host-side prep shared by kernel() and test.py: returns (consts, xr)."""
    x = np.asarray(x)
    B = x.shape[0]
    xr = np.ascontiguousarray(
        x.reshape(B, C * G, HWP).astype(np.float32))
    cU, cV, cA, cCa, cCb, cR1, cR2 = _build_consts()
    wf = _fold_weights(np.asarray(conv_w).astype(np.float64))
    bias = np.ascontiguousarray(
        np.asarray(conv_b).astype(np.float32).reshape(64, 1))
    consts = dict(cU=cU, cV=cV, cA=cA, cCa=cCa, cCb=cCb, cR1=cR1, cR2=cR2,
                  wf=wf, bias=bias)
    return consts, xr


_PROG_CACHE = {}


def _build_program(loop_n=1, unroll=False):
    import concourse.bass as bass
    import concourse.bacc as bacc
    import concourse.tile as tile
    import concourse.mybir as mybir

    f32 = mybir.dt.float32
    f32r = mybir.dt.float32r
    bf16 = mybir.dt.bfloat16
    nc = bacc.Bacc("TRN2", target_bir_lowering=False, debug=False,
                   num_devices=NCORES)

    x_d = nc.dram_tensor("x", [BPC, C * G, HWP], f32r,
                         kind="ExternalInput").ap()
    cU_d = nc.dram_tensor("cU", [128, 112], f32r, kind="ExternalInput").ap()
    cV_d = nc.dram_tensor("cV", [128, 112], f32r, kind="ExternalInput").ap()
    cA_d = nc.dram_tensor("cA", [128, 128], f32r, kind="ExternalInput").ap()
    cCa_d = nc.dram_tensor("cCa", [112, 128], f32r, kind="ExternalInput").ap()
    cCb_d = nc.dram_tensor("cCb", [112, 48], f32r, kind="ExternalInput").ap()
    cR1_d = nc.dram_tensor("cR1", [128, 112], f32r, kind="ExternalInput").ap()
    cR2_d = nc.dram_tensor("cR2", [48, 112], f32r, kind="ExternalInput").ap()
    wf_d = nc.dram_tensor("wf", [112, NQ * 64], f32r,
                          kind="ExternalInput").ap()
    bias_d = nc.dram_tensor("bias", [64, 1], f32, kind="ExternalInput").ap()
    y_d = nc.dram_tensor("y", [BPC, 64, HWP], f32, kind="ExternalOutput").ap()

    LN = mybir.ActivationFunctionType.Ln
    RELU = mybir.ActivationFunctionType.Relu
    IDENT = mybir.ActivationFunctionType.Identity
    MAX = mybir.AluOpType.max

    with tile.TileContext(nc) as tc:
        with (
            tc.tile_pool(name="consts", bufs=1) as cpool,
            tc.tile_pool(name="xin", bufs=10) as xpool,
            tc.tile_pool(name="sb", bufs=4) as sbpool,
            tc.tile_pool(name="rg", bufs=3) as rgpool,
            tc.tile_pool(name="rl", bufs=3) as rlpool,
            tc.tile_pool(name="yout", bufs=2) as ypool,
            tc.tile_pool(name="psA", bufs=1, space="PSUM") as psA,
            tc.tile_pool(name="psB", bufs=1, space="PSUM") as psB,
            tc.tile_pool(name="psC", bufs=1, space="PSUM") as psC,
            tc.tile_pool(name="psY", bufs=2, space="PSUM") as psY,
        ):
            # --- load constants once ---
            cU = cpool.tile([128, 112], f32r, tag="cU")
            cV = cpool.tile([128, 112], f32r, tag="cV")
            cA = cpool.tile([128, 128], f32r, tag="cA")
            cCa = cpool.tile([112, 128], f32r, tag="cCa")
            cCb = cpool.tile([112, 48], f32r, tag="cCb")
            cR1 = cpool.tile([128, 112], f32r, tag="cR1")
            cR2 = cpool.tile([48, 112], f32r, tag="cR2")
            wf = cpool.tile([112, NQ * 64], f32r, tag="wf")
            bias = cpool.tile([64, 1], f32, tag="bias")
            for t, d in [(cU, cU_d), (cV, cV_d), (cA, cA_d), (cCa, cCa_d),
                         (cCb, cCb_d), (cR1, cR1_d), (cR2, cR2_d),
                         (wf, wf_d), (bias, bias_d)]:
                nc.sync.dma_start(out=t[:], in_=d)

            def emit_iter():
              for b in range(BPC):
                  ys = ypool.tile([64, HWP], f32, tag="ys")
                  rgs = rls = None
                  for j in range(NCHUNK):
                      s0 = j * S
                      half = (j % 2) * S
                      last = (j == NCHUNK - 1)
                      if j % 2 == 0:
                          rgs = [rgpool.tile([112, 2 * S], bf16, tag=f"rg{q}",
                                             name=f"rg{q}")
                                 for q in range(NQ)]
                      xts = []
                      for q in range(NQ):
                          xt = xpool.tile([128, S], f32r, tag="x", name="xt")
                          nc.sync.dma_start(
                              out=xt[:], in_=x_d[b, 128 * q:128 * (q + 1),
                                                 s0:s0 + S])
                          xts.append(xt)
                      for q in range(NQ):
                          xt = xts[q]
                          # form matmuls. V and A share a 2-bank PSUM
                          # tile (V at cols 0:448 in bank 0, A at 512:960 in
                          # bank 1) so ONE ScalarE op evacuates both; the
                          # 448:512 pad is stale-but-finite PSUM.
                          pU = psA.tile([112, S], f32, tag="u")
                          pVA = psB.tile([128, 1024], f32, tag="va")
                          nc.tensor.matmul(pU[:], cU[:], xt[:])
                          nc.tensor.matmul(pVA[:112, 0:S], cV[:], xt[:])
                          nc.tensor.matmul(pVA[:, 512:512 + S], cA[:], xt[:])
                          vasb = sbpool.tile([128, 1024], f32, tag="vasb")
                          nc.scalar.copy(vasb[:, 0:512 + S],
                                         pVA[:, 0:512 + S])
                          vsb = vasb[:112, 0:S]
                          asb = vasb[:, 512:512 + S]
                          # round-1 products
                          m1 = sbpool.tile([112, S], f32r, tag="m1")
                          nc.vector.tensor_mul(m1[:], pU[:], vsb)
                          # quadratic combines
                          pCa = psC.tile([128, S], f32, tag="ca")
                          pCb = psC.tile([48, S], f32, tag="cb")
                          nc.tensor.matmul(pCa[:], cCa[:], m1[:])
                          nc.tensor.matmul(pCb[:], cCb[:], m1[:])
                          # round-2 products
                          t1 = sbpool.tile([128, S], f32r, tag="t1")
                          t2 = sbpool.tile([48, S], f32r, tag="t2")
                          nc.vector.tensor_mul(t1[:], asb, pCa[:])
                          nc.vector.tensor_mul(
                              t2[:], vasb[64:112, 512:512 + S], pCb[:])
                          # beta combine
                          pR = psB.tile([112, S], f32, tag="rpre")
                          nc.tensor.matmul(pR[:], cR1[:], t1[:],
                                           start=True, stop=False)
                          nc.tensor.matmul(pR[:], cR2[:], t2[:],
                                           start=False, stop=True)
                          # relu into the pair-chunk slab (2 q's on DVE,
                          # 2 on ACT); ln(1+x) follows per pair
                          if q == 0:
                              nc.vector.tensor_scalar(
                                  rgs[q][:, half:half + S], pR[:],
                                  0.0, None, MAX)
                          else:
                              nc.scalar.activation(
                                  rgs[q][:, half:half + S], pR[:], RELU)
                      if j % 2 == 1 or last:
                          w = half + S
                          rls = [rlpool.tile([112, 2 * S], f32r, tag=f"rl{q}",
                                             name=f"rl{q}")
                                 for q in range(NQ)]
                          for q in range(NQ):
                              nc.scalar.activation(rls[q][:, :w],
                                                   rgs[q][:, :w], LN, bias=1.0)
                          for jj in range(j - (0 if last and j % 2 == 0
                                               else 1), j + 1):
                              ss = jj * S
                              hh = (jj % 2) * S
                              pY = psY.tile([64, S], f32, tag="y")
                              for q in range(NQ):
                                  nc.tensor.matmul(
                                      pY[:], wf[:, q * 64:(q + 1) * 64],
                                      rls[q][:, hh:hh + S],
                                      start=(q == 0), stop=(q == NQ - 1))
                              nc.scalar.activation(ys[:, ss:ss + S], pY[:],
                                                   IDENT, bias=bias[:, 0:1])
                  nc.scalar.dma_start(out=y_d[b], in_=ys[:])

            # Each For_i iteration costs an all-engine barrier; amortize it
            # by emitting 4 bodies per hardware-loop iteration.
            if unroll or loop_n <= 1:
                for _ in range(max(1, loop_n) if not unroll else loop_n):
                    emit_iter()
            else:
                U = 4
                n_for = (loop_n - 1) // U
                rem = loop_n - n_for * U
                for _ in range(rem):
                    emit_iter()
                if n_for > 0:
                    with tc.For_i(0, n_for, 1):
                        for _ in range(U):
                            emit_iter()
    nc.compile()
    return nc


def kernel(x, conv_w, conv_b):
    from concourse.bass_utils import run_bass_kernel_spmd

    consts, xr = _prep(x, conv_w, conv_b)

    key = "prog"
    if key not in _PROG_CACHE:
        _PROG_CACHE[key] = _build_program()
    nc = _PROG_CACHE[key]

    in_maps = []
    for i in range(NCORES):
        m = dict(consts)
        m["x"] = np.ascontiguousarray(xr[i * BPC:(i + 1) * BPC])
        in_maps.append(m)

    res = run_bass_kernel_spmd(nc, in_maps, core_ids=list(range(NCORES)))
    y = np.concatenate([res.results[i]["y"] for i in range(NCORES)], axis=0)
    return np.ascontiguousarray(
        y.reshape(NCORES * BPC, 64, 56, 56).astype(np.float32))


# revision 30
# speedup vs baseline: 1.2074x; 1.0077x over previous
"""Trainium2 Bass kernel for nn_BispectrumPool.

Math (validated vs reference):
  F = FFT_8 along the group axis. beta[k] = F1*F[k]*conj(F[1+k mod 8]).
  Due to conjugate symmetry of the real-input FFT:
    beta4=beta3, beta5=beta2, beta6=beta1, beta7=beta0 (real), Im(beta0)=0
  -> only 7 distinct nonzero features per channel:
     [beta0r, beta1r, beta1i, beta2r, beta2i, beta3r, beta3i]
  with
     beta0r = F0*(b1^2+b2^2)
     beta1  = G*conj(F2),  G = F1^2      (Gr=b1^2-b2^2, Gi=2*b1b2)
     beta2  = F2*H,        H = F1*conj(F3)
     beta3  = F4*K,        K = F1*F3
  (b1,b2)=(Re,Im)F1, (b3,b4)=F2, (b5,b6)=F3, b7=F4(real), b0=F0(real).
  feat = ln(1+relu(beta_part)); y = W_folded @ feat + bias, where the 16
  original feature columns fold onto the 7 distinct ones (cols 8,15 drop).

Distribution: pure data parallel, batch 16 -> 2 per core on 8 cores.

v2 layout: x and form-matmul consts in bf16 (halves DMA); relu writes into
per-(b,q) full-row slabs [112, 3136] (3 of 4 q's on ACT, 1 on DVE); ln runs
once per (b,q) over the whole slab (amortizes ACT fixed costs); conv reads
bf16 slabs; y accumulates into a [64, 3136] slab, one output DMA per batch.
"""

import numpy as np
import ml_dtypes

C, G = 64, 8
HWP = 56 * 56            # 3136
S = 448                  # chunk width (3136 = 7*448)
NCHUNK = HWP // S        # 7
NCORES = 8
BPC = 2                  # batches per core
NQ = 4                   # channel blocks of 16


def _form_rows():
    g = np.arange(G)
    B1 = np.cos(2 * np.pi * g / G)
    B2 = -np.sin(2 * np.pi * g / G)
    B3 = np.cos(4 * np.pi * g / G)
    B4 = -np.sin(4 * np.pi * g / G)
    B5 = np.cos(6 * np.pi * g / G)
    B6 = -np.sin(6 * np.pi * g / G)
    B7 = np.cos(np.pi * g)
    B0 = np.ones(G)
    U = np.stack([B1, B2, B1, B1, B2, B2, B1])            # 7 rows
    V = np.stack([B1, B2, B2, B5, B6, B5, B6])            # 7 rows
    # blocks 4..6 = [b3, b4, b7] so the T2 product can slice at partition 64
    # (SBUF engine access must start at a 32-aligned partition)
    A = np.stack([B0, B3, B4, B3, B3, B4, B7, B4])        # 8 rows
    return U, V, A


def _combine_mats():
    # M1 blocks: [b1^2, b2^2, b1b2, b1b5, b2b6, b2b5, b1b6]
    Wc_a = np.zeros((8, 7))
    Wc_a[0, 0] = Wc_a[0, 1] = 1              # S+
    Wc_a[1, 0], Wc_a[1, 1] = 1, -1           # Gr
    Wc_a[2, 2] = 2.0                         # Gi
    Wc_a[3, 2] = 2.0                         # Gi
    Wc_a[4, 3] = Wc_a[4, 4] = 1              # Hr
    Wc_a[5, 5], Wc_a[5, 6] = 1, -1           # Hi
    Wc_a[6, 3], Wc_a[6, 4] = 1, -1           # Kr
    Wc_a[7, 0], Wc_a[7, 1] = 1, -1           # Gr
    Wc_b = np.zeros((3, 7))
    Wc_b[0, 5], Wc_b[0, 6] = 1, -1           # Hi
    Wc_b[1, 3] = Wc_b[1, 4] = 1              # Hr
    Wc_b[2, 5] = Wc_b[2, 6] = 1              # Ki
    # T1 blocks: [b0S+, b3Gr, b4Gi, b3Gi, b3Hr, b4Hi, b7Kr, b4Gr]
    # T2 blocks: [b3Hi, b4Hr, b7Ki]
    Wr_1 = np.zeros((7, 8))
    Wr_1[0, 0] = 1                            # beta0r
    Wr_1[1, 1] = Wr_1[1, 2] = 1               # beta1r
    Wr_1[2, 3], Wr_1[2, 7] = 1, -1            # beta1i
    Wr_1[3, 4], Wr_1[3, 5] = 1, -1            # beta2r
    Wr_1[5, 6] = 1                            # beta3r
    Wr_2 = np.zeros((7, 3))
    Wr_2[4, 0] = Wr_2[4, 1] = 1               # beta2i
    Wr_2[6, 2] = 1                            # beta3i
    return Wc_a, Wc_b, Wr_1, Wr_2


def _block_diag_lhsT(rows_by_block_out, n_in_blocks, blk=16, in_block_of=None,
                     coef=None):
    """lhsT[k_partition, m] for a block-structured map."""
    n_out = len(coef)
    lhsT = np.zeros((n_in_blocks * blk, n_out * blk), dtype=np.float32)
    for mb in range(n_out):
        for kb in range(n_in_blocks):
            if coef[mb][kb] != 0.0:
                for c in range(blk):
                    lhsT[kb * blk + c, mb * blk + c] = coef[mb][kb]
    return lhsT


def _build_consts():
    U, V, A = _form_rows()
    Wc_a, Wc_b, Wr_1, Wr_2 = _combine_mats()

    # form matmuls: input partitions = (16c x 8g), c-major.
    def form_lhsT(rows):
        n_out = rows.shape[0]
        lhsT = np.zeros((128, n_out * 16), dtype=np.float32)
        for j in range(n_out):
            for c in range(16):
                for g in range(G):
                    lhsT[c * G + g, j * 16 + c] = rows[j, g]
        return lhsT

    cU = form_lhsT(U)                                 # [128, 112]
    cV = form_lhsT(V)                                 # [128, 112]
    cA = form_lhsT(A)                                 # [128, 128]
    cCa = _block_diag_lhsT(None, 7, coef=Wc_a).astype(np.float32)   # [112, 128]
    cCb = _block_diag_lhsT(None, 7, coef=Wc_b).astype(np.float32)   # [112, 48]
    cR1 = _block_diag_lhsT(None, 8, coef=Wr_1).astype(np.float32)   # [128, 112]
    cR2 = _block_diag_lhsT(None, 3, coef=Wr_2).astype(np.float32)   # [48, 112]
    return cU, cV, cA, cCa, cCb, cR1, cR2


def _fold_weights(conv_w):
    w = conv_w.reshape(64, C, 16)
    W7 = np.zeros((64, C, 7), dtype=np.float64)
    W7[..., 0] = w[..., 0] + w[..., 7]
    W7[..., 1] = w[..., 1] + w[..., 6]
    W7[..., 2] = w[..., 9] + w[..., 14]
    W7[..., 3] = w[..., 2] + w[..., 5]
    W7[..., 4] = w[..., 10] + w[..., 13]
    W7[..., 5] = w[..., 3] + w[..., 4]
    W7[..., 6] = w[..., 11] + w[..., 12]
    # conv lhsT per q: [112 = (7f x 16c), 64], packed side by side -> [112, 256]
    wf = np.zeros((112, NQ * 64), dtype=np.float32)
    for q in range(NQ):
        for f in range(7):
            for cl in range(16):
                wf[f * 16 + cl, q * 64:(q + 1) * 64] = W7[:, q * 16 + cl, f]
    return wf


def _prep(x, conv_w, conv_b):
    """Host-side prep shared by kernel() and test.py: returns (consts, xr)."""
    x = np.asarray(x)
    B = x.shape[0]
    xr = np.ascontiguousarray(
        x.reshape(B, C * G, HWP).astype(np.float32))
    cU, cV, cA, cCa, cCb, cR1, cR2 = _build_consts()
    wf = _fold_weights(np.asarray(conv_w).astype(np.float64))
    bias = np.ascontiguousarray(
        np.asarray(conv_b).astype(np.float32).reshape(64, 1))
    consts = dict(cU=cU, cV=cV, cA=cA, cCa=cCa, cCb=cCb, cR1=cR1, cR2=cR2,
                  wf=wf, bias=bias)
    return consts, xr


_PROG_CACHE = {}


def _build_program(loop_n=1, unroll=False):
    import concourse.bass as bass
    import concourse.bacc as bacc
    import concourse.tile as tile
    import concourse.mybir as mybir

    f32 = mybir.dt.float32
    f32r = mybir.dt.float32r
    bf16 = mybir.dt.bfloat16
    nc = bacc.Bacc("TRN2", target_bir_lowering=False, debug=False,
                   num_devices=NCORES)

    x_d = nc.dram_tensor("x", [BPC, C * G, HWP], f32r,
                         kind="ExternalInput").ap()
    cU_d = nc.dram_tensor("cU", [128, 112], f32r, kind="ExternalInput").ap()
    cV_d = nc.dram_tensor("cV", [128, 112], f32r, kind="ExternalInput").ap()
    cA_d = nc.dram_tensor("cA", [128, 128], f32r, kind="ExternalInput").ap()
    cCa_d = nc.dram_tensor("cCa", [112, 128], f32r, kind="ExternalInput").ap()
    cCb_d = nc.dram_tensor("cCb", [112, 48], f32r, kind="ExternalInput").ap()
    cR1_d = nc.dram_tensor("cR1", [128, 112], f32r, kind="ExternalInput").ap()
    cR2_d = nc.dram_tensor("cR2", [48, 112], f32r, kind="ExternalInput").ap()
    wf_d = nc.dram_tensor("wf", [112, NQ * 64], f32r,
                          kind="ExternalInput").ap()
    bias_d = nc.dram_tensor("bias", [64, 1], f32, kind="ExternalInput").ap()
    y_d = nc.dram_tensor("y", [BPC, 64, HWP], f32, kind="ExternalOutput").ap()

    LN = mybir.ActivationFunctionType.Ln
    RELU = mybir.ActivationFunctionType.Relu
    IDENT = mybir.ActivationFunctionType.Identity
    MAX = mybir.AluOpType.max

    with tile.TileContext(nc) as tc:
        with (
            tc.tile_pool(name="consts", bufs=1) as cpool,
            tc.tile_pool(name="xin", bufs=10) as xpool,
            tc.tile_pool(name="sb", bufs=4) as sbpool,
            tc.tile_pool(name="rg", bufs=3) as rgpool,
            tc.tile_pool(name="rl", bufs=3) as rlpool,
            tc.tile_pool(name="yout", bufs=2) as ypool,
            tc.tile_pool(name="psA", bufs=1, space="PSUM") as psA,
            tc.tile_pool(name="psB", bufs=1, space="PSUM") as psB,
            tc.tile_pool(name="psC", bufs=1, space="PSUM") as psC,
            tc.tile_pool(name="psY", bufs=2, space="PSUM") as psY,
        ):
            # --- load constants once ---
            cU = cpool.tile([128, 112], f32r, tag="cU")
            cV = cpool.tile([128, 112], f32r, tag="cV")
            cA = cpool.tile([128, 128], f32r, tag="cA")
            cCa = cpool.tile([112, 128], f32r, tag="cCa")
            cCb = cpool.tile([112, 48], f32r, tag="cCb")
            cR1 = cpool.tile([128, 112], f32r, tag="cR1")
            cR2 = cpool.tile([48, 112], f32r, tag="cR2")
            wf = cpool.tile([112, NQ * 64], f32r, tag="wf")
            bias = cpool.tile([64, 1], f32, tag="bias")
            for t, d in [(cU, cU_d), (cV, cV_d), (cA, cA_d), (cCa, cCa_d),
                         (cCb, cCb_d), (cR1, cR1_d), (cR2, cR2_d),
                         (wf, wf_d), (bias, bias_d)]:
                nc.sync.dma_start(out=t[:], in_=d)

            def emit_iter():
              for b in range(BPC):
                  ys = ypool.tile([64, HWP], f32, tag="ys")
                  rgs = rls = None
                  for j in range(NCHUNK):
                      s0 = j * S
                      half = (j % 2) * S
                      last = (j == NCHUNK - 1)
                      if j % 2 == 0:
                          rgs = [rgpool.tile([112, 2 * S], bf16, tag=f"rg{q}",
                                             name=f"rg{q}")
                                 for q in range(NQ)]
                      xts = []
                      for q in range(NQ):
                          xt = xpool.tile([128, S], f32r, tag="x", name="xt")
                          nc.sync.dma_start(
                              out=xt[:], in_=x_d[b, 128 * q:128 * (q + 1),
                                                 s0:s0 + S])
                          xts.append(xt)
                      for q in range(NQ):
                          xt = xts[q]
                          # form matmuls. V and A share a 2-bank PSUM
                          # tile (V at cols 0:448 in bank 0, A at 512:960 in
                          # bank 1) so ONE ScalarE op evacuates both; the
                          # 448:512 pad is stale-but-finite PSUM.
                          pU = psA.tile([112, S], f32, tag="u")
                          pVA = psB.tile([128, 1024], f32, tag="va")
                          nc.tensor.matmul(pU[:], cU[:], xt[:])
                          nc.tensor.matmul(pVA[:112, 0:S], cV[:], xt[:])
                          nc.tensor.matmul(pVA[:, 512:512 + S], cA[:], xt[:])
                          vasb = sbpool.tile([128, 1024], f32, tag="vasb")
                          nc.scalar.copy(vasb[:, 0:512 + S],
                                         pVA[:, 0:512 + S])
                          vsb = vasb[:112, 0:S]
                          asb = vasb[:, 512:512 + S]
                          # round-1 products
                          m1 = sbpool.tile([112, S], f32r, tag="m1")
                          nc.vector.tensor_mul(m1[:], pU[:], vsb)
                          # quadratic combines
                          pCa = psC.tile([128, S], f32, tag="ca")
                          pCb = psC.tile([48, S], f32, tag="cb")
                          nc.tensor.matmul(pCa[:], cCa[:], m1[:])
                          nc.tensor.matmul(pCb[:], cCb[:], m1[:])
                          # round-2 products
                          t1 = sbpool.tile([128, S], f32r, tag="t1")
                          t2 = sbpool.tile([48, S], f32r, tag="t2")
                          nc.vector.tensor_mul(t1[:], asb, pCa[:])
                          nc.vector.tensor_mul(
                              t2[:], vasb[64:112, 512:512 + S], pCb[:])
                          # beta combine
                          pR = psB.tile([112, S], f32, tag="rpre")
                          nc.tensor.matmul(pR[:], cR1[:], t1[:],
                                           start=True, stop=False)
                          nc.tensor.matmul(pR[:], cR2[:], t2[:],
                                           start=False, stop=True)
                          # relu into the pair-chunk slab (2 q's on DVE,
                          # 2 on ACT); ln(1+x) follows per pair
                          if q == 0:
                              nc.vector.tensor_scalar(
                                  rgs[q][:, half:half + S], pR[:],
                                  0.0, None, MAX)
                          else:
                              nc.scalar.activation(
                                  rgs[q][:, half:half + S], pR[:], RELU)
                      if j % 2 == 1 or last:
                          w = half + S
                          rls = [rlpool.tile([112, 2 * S], f32r, tag=f"rl{q}",
                                             name=f"rl{q}")
                                 for q in range(NQ)]
                          for q in range(NQ):
                              nc.scalar.activation(rls[q][:, :w],
                                                   rgs[q][:, :w], LN, bias=1.0)
                          for jj in range(j - (0 if last and j % 2 == 0
                                               else 1), j + 1):
                              ss = jj * S
                              hh = (jj % 2) * S
                              pY = psY.tile([64, S], f32, tag="y")
                              for q in range(NQ):
                                  nc.tensor.matmul(
                                      pY[:], wf[:, q * 64:(q + 1) * 64],
                                      rls[q][:, hh:hh + S],
                                      start=(q == 0), stop=(q == NQ - 1))
                              nc.scalar.activation(ys[:, ss:ss + S], pY[:],
                                                   IDENT, bias=bias[:, 0:1])
                  nc.scalar.dma_start(out=y_d[b], in_=ys[:])

            # Each For_i iteration costs an all-engine barrier; amortize it
            # by emitting 4 bodies per hardware-loop iteration.
            if unroll or loop_n <= 1:
                for _ in range(max(1, loop_n) if not unroll else loop_n):
                    emit_iter()
            else:
                U = 16
                n_for = (loop_n - 1) // U
                rem = loop_n - n_for * U
                for _ in range(rem):
                    emit_iter()
                if n_for > 0:
                    with tc.For_i(0, n_for, 1):
                        for _ in range(U):
                            emit_iter()
    nc.compile()
    return nc


def kernel(x, conv_w, conv_b):
    from concourse.bass_utils import run_bass_kernel_spmd

    consts, xr = _prep(x, conv_w, conv_b)

    key = "prog"
    if key not in _PROG_CACHE:
        _PROG_CACHE[key] = _build_program()
    nc = _PROG_CACHE[key]

    in_maps = []
    for i in range(NCORES):
        m = dict(consts)
        m["x"] = np.ascontiguousarray(xr[i * BPC:(i + 1) * BPC])
        in_maps.append(m)

    res = run_bass_kernel_spmd(nc, in_maps, core_ids=list(range(NCORES)))
    y = np.concatenate([res.results[i]["y"] for i in range(NCORES)], axis=0)
    return np.ascontiguousarray(
        y.reshape(NCORES * BPC, 64, 56, 56).astype(np.float32))
